# revision 36
# baseline (speedup 1.0000x reference)
"""MoE MLP (E=32 experts, top-2, D=H=1024) on 8 Trainium2 NeuronCores.

Strategy (expert parallel, per sharding hint):
  * Host computes the (tiny) gate: softmax(x @ Wg), top-2, renormalized
    weights, and dispatches tokens per expert into per-expert token blocks,
    transposed to [D, tokens] (features on SBUF partitions, tokens on the
    matmul moving/free dimension). This is the sharding/all-to-all step.
  * Experts are assigned to cores in "snake" order of descending token
    count, so every core holds 4 experts whose block sizes match the
    per-position maximum; blocks are sized to the actual routed counts
    (rounded up to 8) instead of a uniform worst-case capacity.  SPMD
    requires one program for all cores, so position k on every core uses
    the same block size s_k = max over cores of that position's count.
  * Each core computes GELU(x W1 + b1) W2 + b2 for its experts' blocks.
  * Host combines with the top-2 gate weights (scatter-add).

Device kernel notes:
  * Weights are host-pre-tiled to [e, col_tile, partition, k_tile, 128] so
    each chunk streams in as one fully-contiguous DMA transfer.
  * All input transfers (x halves, W1 quarters, W2 halves) form one stream
    in exact consumption order, ping-ponged across the sync HWDGE ring and
    the gpsimd SWDGE queue with bounded (bufs) lookahead, so delivery
    tracks the need order at aggregate HBM rate (~345 GB/s/core, the
    binding constraint — this problem sits right at the compute/memory
    ridge). The scalar ring carries only output stores: weight triggers
    there would queue behind that expert's ACTs and lose all prefetch
    lookahead.
  * Output is written per half-expert in bf16, so the post-matmul tail
    (bias add + store) is short and overlaps the next tile's matmuls.
  * A chain of N=512 dummy matmuls at kernel start bridges the PE from
    t~7.5us until the first weight/x chunks land (~12.5us), keeping the
    HAM clock-gate warm through the handoff to real matmuls.
"""

import os
import sys
import numpy as np

for _p in ("/root/.axon_site/_ro/trn_rl_repo", "/opt/trn_rl_repo"):
    if _p not in sys.path and os.path.isdir(_p):
        sys.path.append(_p)

E, D, H = 32, 1024, 1024
TOP_K = 2
N_CORES = 8
EPC = E // N_CORES  # experts per core
ND = D // 128       # d 128-tiles
NH = H // 128       # h 128-tiles

# weight dtype, activation dtype (must both be 16-bit or both 32-bit)
DT_W = os.environ.get("MOE_DT_W", "bfloat16")
DT_A = os.environ.get("MOE_DT_A", "bfloat16")
DT_Y = os.environ.get("MOE_DT_Y", "bfloat16")
N_WARMUP_MM = int(os.environ.get("MOE_WARMUP", "16"))
WARMUP_N = int(os.environ.get("MOE_WARMUP_N", "512"))
WBUFS = int(os.environ.get("MOE_WBUFS", "6"))

LAST_EXEC_TIME_NS = None

_NC_CACHE = {}


def _build_nc(sizes, dt_w_name, dt_a_name, dt_y_name):
    import concourse.bass as bass  # noqa: F401
    import concourse.tile as tile
    from concourse import bacc, mybir
    from contextlib import ExitStack

    f32 = mybir.dt.float32
    dt_w = getattr(mybir.dt, dt_w_name)
    dt_a = getattr(mybir.dt, dt_a_name)
    dt_y = getattr(mybir.dt, dt_y_name)
    S = sum(sizes)

    nc = bacc.Bacc(
        "TRN2",
        target_bir_lowering=False,
        debug=False,
        enable_asserts=False,
        num_devices=N_CORES,
    )
    xT = nc.dram_tensor("xT", [D, S], dt_a, kind="ExternalInput").ap()
    # host-pre-tiled: w1[e, ht, p(=d_in), dt, hi], w2[e, dt, p(=h_in), ht, di]
    w1 = nc.dram_tensor("w1", [EPC, NH, 128, ND, 128], dt_w, kind="ExternalInput").ap()
    w2 = nc.dram_tensor("w2", [EPC, ND, 128, NH, 128], dt_w, kind="ExternalInput").ap()
    # host-pre-transposed biases: [p, e, col_tile]
    b1 = nc.dram_tensor("b1", [128, EPC, NH], f32, kind="ExternalInput").ap()
    b2 = nc.dram_tensor("b2", [128, EPC, ND], f32, kind="ExternalInput").ap()
    yT = nc.dram_tensor("yT", [D, S], dt_y, kind="ExternalOutput").ap()

    with tile.TileContext(nc) as tc, ExitStack() as ctx:
        wpool = ctx.enter_context(tc.tile_pool(name="w", bufs=3))
        xpool = ctx.enter_context(tc.tile_pool(name="x", bufs=EPC))
        hpool = ctx.enter_context(tc.tile_pool(name="h", bufs=NH))
        ypool = ctx.enter_context(tc.tile_pool(name="y", bufs=2))
        bpool = ctx.enter_context(tc.tile_pool(name="b", bufs=1))
        pp1 = ctx.enter_context(tc.tile_pool(name="ps1", bufs=4, space="PSUM"))
        pp2 = ctx.enter_context(tc.tile_pool(name="ps2", bufs=3, space="PSUM"))
        ppw = ctx.enter_context(tc.tile_pool(name="psw", bufs=1, space="PSUM"))

        # PE warm-up: dummy matmuls with no DMA dependency keep the PE
        # busy from t~0 so HAM un-throttles before the real matmuls.
        if N_WARMUP_MM:
            wu = bpool.tile([128, WARMUP_N], mybir.dt.bfloat16, tag="wu")
            nc.vector.memset(wu[:], 0.0)
            wups = ppw.tile([128, WARMUP_N], f32, tag="psw")
            for i in range(N_WARMUP_MM):
                nc.tensor.matmul(wups[:], wu[:, :128], wu[:],
                                 start=(i == 0), stop=(i == N_WARMUP_MM - 1))

        gelu = mybir.ActivationFunctionType.Gelu
        S0 = sizes[0]  # max block size; all tiles sized for it, sliced to C
        offs = [sum(sizes[:k]) for k in range(EPC)]

        # Biases first on gpsimd — tiny transfers that absorb the SWDGE
        # queue's ~2us descriptor-emission cold-start before its first
        # weight chunk (measured: moving them off gpsimd delays W1e0's
        # gpsimd-side quarters and stalls L1(e0) by ~3us).
        b1_sb = bpool.tile([128, EPC * NH], f32, tag="b1")
        b2_sb = bpool.tile([128, EPC * ND], f32, tag="b2")
        nc.gpsimd.dma_start(
            out=b1_sb[:].rearrange("p (e ht) -> p e ht", e=EPC), in_=b1[:])
        nc.gpsimd.dma_start(
            out=b2_sb[:].rearrange("p (e dt) -> p e dt", e=EPC), in_=b2[:])

        # All input transfers (x and weight chunks) form ONE stream in exact
        # consumption order, split across the sync HWDGE ring and the gpsimd
        # SWDGE queue by greedy byte-balancing weighted with each queue's
        # measured service rate (SWDGE drains ~1.3x faster when both are
        # saturated), so both rings finish each need-order window together.
        # The scalar ring carries only output stores: weight triggers there
        # would queue behind that expert's ACTs and lose prefetch lookahead.
        ring = [nc.sync, nc.gpsimd]
        g = [0]

        def stream_dma(out_ap, in_ap, nbytes):
            # First three transfers (x0 halves, W1e0 q0) ride sync: the
            # SWDGE queue's cold start would otherwise gate the first MMs.
            eng = nc.sync if g[0] < 3 else ring[g[0] % 2]
            g[0] += 1
            eng.dma_start(out=out_ap, in_=in_ap)

        HD = ND // 2

        def xload_half(e, h):
            # x in two dt-halves so the first L1 accumulation group can
            # start before the whole x tile has landed. x0 rides the sync
            # prefix; x1..x3 ride the scalar ring, which is otherwise idle
            # until the first stores (~21us) — this takes 1.6MB off the
            # weight streams and shrinks the early HBM deficit. bufs=EPC
            # keeps every x tile live so no x trigger ever waits on buffer
            # recycling (a waiting trigger would block the scalar FIFO
            # ahead of the ACTs it feeds -> deadlock).
            C = sizes[e]
            xt = xpool.tile([128, HD * S0], dt_a, tag=f"xt{h}", bufs=EPC)
            out_ap = xt[:, :HD * C].rearrange("p (dt t) -> p dt t", dt=HD)
            in_ap = (xT[h * HD * 128:(h + 1) * HD * 128, offs[e]:offs[e] + C]
                     .rearrange("(dt p) t -> p dt t", p=128))
            if e == 0:
                stream_dma(out_ap, in_ap, HD * C * 128 * 2)
            else:
                nc.scalar.dma_start(out=out_ap, in_=in_ap)
            return xt

        def wchunk(shape, tag, bufs, out_pat, in_ap):
            wt = wpool.tile(shape, dt_w, tag=tag, bufs=bufs)
            stream_dma(wt[:].rearrange(out_pat[0], **out_pat[1]), in_ap,
                       shape[0] * shape[1] * 2)
            return wt

        HND = ND // 2
        csz = NH // 4

        def w1chunk(e, q):
            # W1 quarters (fine grain so each L1 ht-group waits only for its
            # own 512KB slice; DMA-paced slips stay well under the HAM
            # re-throttle window)
            return wchunk(
                [128, csz * ND * 128], "w1q", 8,
                ("p (ht dt hi) -> p ht dt hi", dict(ht=csz, dt=ND)),
                w1[e, q * csz:(q + 1) * csz].rearrange("ht p dt hi -> p ht dt hi"),
            )

        xts = {}
        w1q, w2q = {}, {}
        # expert 0: interleave x halves with the first W1 quarters so the
        # first matmuls are gated on the minimum number of bytes.
        xts[0] = [None, None]
        xts[0][0] = xload_half(0, 0)
        w1q[0] = [w1chunk(0, 0)]
        xts[0][1] = xload_half(0, 1)
        w1q[0] += [w1chunk(0, q) for q in range(1, 4)]
        for e in range(EPC):
            if e > 0:
                w1q[e] = [w1chunk(e, q) for q in range(4)]
            w2q[e] = [wchunk(
                [128, HND * NH * 128], "w2h", 4,
                ("p (dt ht di) -> p dt ht di", dict(dt=HND, ht=NH)),
                w2[e, h * HND:(h + 1) * HND].rearrange("dt p ht di -> p dt ht di"),
            ) for h in range(2)]
            if e + 1 < EPC:
                xts[e + 1] = [xload_half(e + 1, 0), xload_half(e + 1, 1)]

        for e in range(EPC):
            C = sizes[e]
            off = offs[e]
            xt = xts[e]
            csz = NH // 4
            w1h = w1q[e]
            w2h = w2q[e]

            hts = []
            for ht in range(NH):
                wt = w1h[ht // csz]
                hoff = (ht % csz) * ND * 128
                ps = pp1.tile([128, S0], f32, tag="ps1")
                for dt_i in range(ND):
                    nc.tensor.matmul(
                        ps[:, :C],
                        wt[:, hoff + dt_i * 128: hoff + (dt_i + 1) * 128],
                        xt[dt_i // HD][:, (dt_i % HD) * C:(dt_i % HD + 1) * C],
                        start=(dt_i == 0),
                        stop=(dt_i == ND - 1),
                    )
                hsb = hpool.tile([128, S0], dt_a, tag="ht")
                nc.scalar.activation(
                    hsb[:, :C], ps[:, :C], gelu,
                    bias=b1_sb[:, e * NH + ht: e * NH + ht + 1],
                )
                hts.append(hsb)
            ysb = ypool.tile([128, ND * S0], dt_y, tag="yt")
            for dt_i in range(ND):
                wt = w2h[dt_i // HND]
                doff = (dt_i % HND) * NH * 128
                ps2 = pp2.tile([128, S0], f32, tag="ps2")
                for ht in range(NH):
                    nc.tensor.matmul(
                        ps2[:, :C],
                        wt[:, doff + ht * 128: doff + (ht + 1) * 128],
                        hts[ht][:, :C],
                        start=(ht == 0),
                        stop=(ht == NH - 1),
                    )
                nc.vector.tensor_scalar_add(
                    ysb[:, dt_i * C:(dt_i + 1) * C], ps2[:, :C],
                    b2_sb[:, e * ND + dt_i: e * ND + dt_i + 1],
                )
                if dt_i % HND == HND - 1:
                    qi = dt_i // HND
                    r0, r1 = qi * HND * 128, (qi + 1) * HND * 128
                    nc.scalar.dma_start(
                        out=yT[r0:r1, off:off + C]
                        .rearrange("(dt p) t -> p dt t", p=128),
                        in_=ysb[:, qi * HND * C:(qi + 1) * HND * C]
                        .rearrange("p (dt t) -> p dt t", dt=HND),
                    )
    nc.compile()
    return nc


def _get_nc(sizes, dt_w, dt_a, dt_y):
    key = (sizes, dt_w, dt_a, dt_y)
    if key not in _NC_CACHE:
        _NC_CACHE[key] = _build_nc(sizes, dt_w, dt_a, dt_y)
    return _NC_CACHE[key]


def _np_dt(name):
    if name == "bfloat16":
        import ml_dtypes
        return np.dtype(ml_dtypes.bfloat16)
    return np.dtype(np.float32)


def _route(xf, Wg):
    """Replicates the reference gate exactly in f32 numpy."""
    logits = xf @ Wg                                     # [T, E]
    m = logits.max(-1, keepdims=True)
    ex = np.exp(logits - m)
    scores = ex / ex.sum(-1, keepdims=True)
    idx = np.argsort(-scores, axis=1, kind="stable")[:, :TOP_K]  # [T, k]
    tw = np.take_along_axis(scores, idx, 1)
    m2 = tw.max(-1, keepdims=True)
    e2 = np.exp(tw - m2)
    w = (e2 / e2.sum(-1, keepdims=True)).astype(np.float32)
    return idx.astype(np.int64), w


def kernel(x, Wg, W1, b1, W2, b2):
    global LAST_EXEC_TIME_NS
    from concourse import bass_utils

    dt_w, dt_a, dt_y = DT_W, DT_A, DT_Y
    orig_shape = x.shape
    x = np.asarray(x, dtype=np.float32)
    Wg = np.asarray(Wg, dtype=np.float32)
    W1 = np.asarray(W1, dtype=np.float32)
    b1 = np.asarray(b1, dtype=np.float32)
    W2 = np.asarray(W2, dtype=np.float32)
    b2 = np.asarray(b2, dtype=np.float32)
    xf = np.ascontiguousarray(x.reshape(-1, D))
    T = xf.shape[0]

    idx, w = _route(xf, Wg)

    # ---- dispatch: snake-balanced expert->core assignment, exact sizes
    counts = np.bincount(idx.reshape(-1), minlength=E)
    rank = np.argsort(-counts, kind="stable")             # expert ids, desc count
    core_of = np.zeros(E, np.int64)
    pos_of = np.zeros(E, np.int64)
    expert_at = np.zeros((N_CORES, EPC), np.int64)        # [core, pos] -> expert
    for k in range(EPC):
        for c in range(N_CORES):
            e = rank[k * N_CORES + (c if k % 2 == 0 else N_CORES - 1 - c)]
            core_of[e] = c
            pos_of[e] = k
            expert_at[c, k] = e
    sizes = tuple(
        int(-(-max(counts[expert_at[c, k]] for c in range(N_CORES)) // 8) * 8)
        for k in range(EPC)
    )
    offs = np.zeros(EPC, np.int64)
    for k in range(1, EPC):
        offs[k] = offs[k - 1] + sizes[k - 1]
    S = int(offs[-1] + sizes[-1])

    flat_e = idx.reshape(-1)                 # [k*T]
    flat_t = np.repeat(np.arange(T), TOP_K)
    order = np.argsort(flat_e, kind="stable")
    starts = np.zeros(E + 1, np.int64)
    starts[1:] = np.cumsum(counts)
    se = flat_e[order]
    pos = np.arange(TOP_K * T) - starts[se]  # slot within expert block
    core = core_of[se]
    col = offs[pos_of[se]] + pos             # column in that core's xT
    tok = flat_t[order]

    gidx = np.zeros((N_CORES, S), np.int64)
    for c in range(N_CORES):
        msel = core == c
        gidx[c, col[msel]] = tok[msel]

    np_w = _np_dt(dt_w)
    np_a = _np_dt(dt_a)
    xf_a = xf.astype(np_a, copy=False)
    # pre-tile weights: w1 -> [e, ht, p(d_in), dt, hi], w2 -> [e, dt, p(h_in), ht, di]
    W1t = np.ascontiguousarray(
        W1.reshape(E, ND, 128, NH, 128).transpose(0, 3, 2, 1, 4).astype(np_w, copy=False))
    W2t = np.ascontiguousarray(
        W2.reshape(E, NH, 128, ND, 128).transpose(0, 3, 2, 1, 4).astype(np_w, copy=False))
    # pre-transpose biases to [p, e, col_tile]
    b1t = np.ascontiguousarray(b1.reshape(E, NH, 128).transpose(2, 0, 1))
    b2t = np.ascontiguousarray(b2.reshape(E, ND, 128).transpose(2, 0, 1))

    in_maps = []
    for c in range(N_CORES):
        es = expert_at[c]
        in_maps.append({
            "xT": np.ascontiguousarray(xf_a[gidx[c]].T),
            "w1": np.ascontiguousarray(W1t[es]),
            "w2": np.ascontiguousarray(W2t[es]),
            "b1": np.ascontiguousarray(b1t[:, es]),
            "b2": np.ascontiguousarray(b2t[:, es]),
        })

    nc = _get_nc(sizes, dt_w, dt_a, dt_y)
    trace = os.environ.get("MOE_TRACE", "0") == "1"
    res = bass_utils.run_bass_kernel_spmd(
        nc, in_maps, core_ids=list(range(N_CORES)), trace=trace,
    )
    LAST_EXEC_TIME_NS = res.exec_time_ns

    # ---- combine: gather each (token, k) contribution, weight, and sum
    Ystack = np.stack(
        [res.results[c]["yT"].T.astype(np.float32) for c in range(N_CORES)])
    contrib = Ystack[core, col]              # [k*T, D] (sorted order)
    inv = np.empty_like(order)
    inv[order] = np.arange(TOP_K * T)
    contrib = contrib[inv].reshape(T, TOP_K, D)
    y = (contrib * w[:, :, None]).sum(1).astype(np.float32)
    return y.reshape(orig_shape)


# revision 38
# speedup vs baseline: 1.0025x; 1.0025x over previous
"""MoE MLP (E=32 experts, top-2, D=H=1024) on 8 Trainium2 NeuronCores.

Strategy (expert parallel, per sharding hint):
  * Host computes the (tiny) gate: softmax(x @ Wg), top-2, renormalized
    weights, and dispatches tokens per expert into per-expert token blocks,
    transposed to [D, tokens] (features on SBUF partitions, tokens on the
    matmul moving/free dimension). This is the sharding/all-to-all step.
  * Experts are assigned to cores in "snake" order of descending token
    count, so every core holds 4 experts whose block sizes match the
    per-position maximum; blocks are sized to the actual routed counts
    (rounded up to 8) instead of a uniform worst-case capacity.  SPMD
    requires one program for all cores, so position k on every core uses
    the same block size s_k = max over cores of that position's count.
  * Each core computes GELU(x W1 + b1) W2 + b2 for its experts' blocks.
  * Host combines with the top-2 gate weights (scatter-add).

Device kernel notes:
  * Weights are host-pre-tiled to [e, col_tile, partition, k_tile, 128] so
    each chunk streams in as one fully-contiguous DMA transfer.
  * All input transfers (x halves, W1 quarters, W2 halves) form one stream
    in exact consumption order, ping-ponged across the sync HWDGE ring and
    the gpsimd SWDGE queue with bounded (bufs) lookahead, so delivery
    tracks the need order at aggregate HBM rate (~345 GB/s/core, the
    binding constraint — this problem sits right at the compute/memory
    ridge). The scalar ring carries only output stores: weight triggers
    there would queue behind that expert's ACTs and lose all prefetch
    lookahead.
  * Output is written per half-expert in bf16, so the post-matmul tail
    (bias add + store) is short and overlaps the next tile's matmuls.
  * A chain of N=512 dummy matmuls at kernel start bridges the PE from
    t~7.5us until the first weight/x chunks land (~12.5us), keeping the
    HAM clock-gate warm through the handoff to real matmuls.
"""

import os
import sys
import numpy as np

for _p in ("/root/.axon_site/_ro/trn_rl_repo", "/opt/trn_rl_repo"):
    if _p not in sys.path and os.path.isdir(_p):
        sys.path.append(_p)

E, D, H = 32, 1024, 1024
TOP_K = 2
N_CORES = 8
EPC = E // N_CORES  # experts per core
ND = D // 128       # d 128-tiles
NH = H // 128       # h 128-tiles

# weight dtype, activation dtype (must both be 16-bit or both 32-bit)
DT_W = os.environ.get("MOE_DT_W", "bfloat16")
DT_A = os.environ.get("MOE_DT_A", "bfloat16")
DT_Y = os.environ.get("MOE_DT_Y", "bfloat16")
N_WARMUP_MM = int(os.environ.get("MOE_WARMUP", "16"))
WARMUP_N = int(os.environ.get("MOE_WARMUP_N", "512"))
WBUFS = int(os.environ.get("MOE_WBUFS", "6"))

LAST_EXEC_TIME_NS = None

_NC_CACHE = {}


def _build_nc(sizes, dt_w_name, dt_a_name, dt_y_name):
    import concourse.bass as bass  # noqa: F401
    import concourse.tile as tile
    from concourse import bacc, mybir
    from contextlib import ExitStack

    f32 = mybir.dt.float32
    dt_w = getattr(mybir.dt, dt_w_name)
    dt_a = getattr(mybir.dt, dt_a_name)
    dt_y = getattr(mybir.dt, dt_y_name)
    S = sum(sizes)

    nc = bacc.Bacc(
        "TRN2",
        target_bir_lowering=False,
        debug=False,
        enable_asserts=False,
        num_devices=N_CORES,
    )
    xT = nc.dram_tensor("xT", [D, S], dt_a, kind="ExternalInput").ap()
    # host-pre-tiled: w1[e, ht, p(=d_in), dt, hi], w2[e, dt, p(=h_in), ht, di]
    w1 = nc.dram_tensor("w1", [EPC, NH, 128, ND, 128], dt_w, kind="ExternalInput").ap()
    w2 = nc.dram_tensor("w2", [EPC, ND, 128, NH, 128], dt_w, kind="ExternalInput").ap()
    # host-pre-transposed biases: [p, e, col_tile]
    b1 = nc.dram_tensor("b1", [128, EPC, NH], f32, kind="ExternalInput").ap()
    b2 = nc.dram_tensor("b2", [128, EPC, ND], f32, kind="ExternalInput").ap()
    yT = nc.dram_tensor("yT", [D, S], dt_y, kind="ExternalOutput").ap()

    with tile.TileContext(nc) as tc, ExitStack() as ctx:
        wpool = ctx.enter_context(tc.tile_pool(name="w", bufs=3))
        xpool = ctx.enter_context(tc.tile_pool(name="x", bufs=EPC))
        hpool = ctx.enter_context(tc.tile_pool(name="h", bufs=NH))
        ypool = ctx.enter_context(tc.tile_pool(name="y", bufs=2))
        bpool = ctx.enter_context(tc.tile_pool(name="b", bufs=1))
        pp1 = ctx.enter_context(tc.tile_pool(name="ps1", bufs=4, space="PSUM"))
        pp2 = ctx.enter_context(tc.tile_pool(name="ps2", bufs=3, space="PSUM"))
        ppw = ctx.enter_context(tc.tile_pool(name="psw", bufs=1, space="PSUM"))

        # PE warm-up: dummy matmuls with no DMA dependency keep the PE
        # busy from t~0 so HAM un-throttles before the real matmuls.
        if N_WARMUP_MM:
            wu = bpool.tile([128, WARMUP_N], mybir.dt.bfloat16, tag="wu")
            nc.vector.memset(wu[:], 0.0)
            wups = ppw.tile([128, WARMUP_N], f32, tag="psw")
            for i in range(N_WARMUP_MM):
                nc.tensor.matmul(wups[:], wu[:, :128], wu[:],
                                 start=(i == 0), stop=(i == N_WARMUP_MM - 1))

        gelu = mybir.ActivationFunctionType.Gelu
        S0 = sizes[0]  # max block size; all tiles sized for it, sliced to C
        offs = [sum(sizes[:k]) for k in range(EPC)]

        # Biases first on gpsimd — tiny transfers that absorb the SWDGE
        # queue's ~2us descriptor-emission cold-start before its first
        # weight chunk (measured: moving them off gpsimd delays W1e0's
        # gpsimd-side quarters and stalls L1(e0) by ~3us).
        b1_sb = bpool.tile([128, EPC * NH], f32, tag="b1")
        b2_sb = bpool.tile([128, EPC * ND], f32, tag="b2")
        nc.gpsimd.dma_start(
            out=b1_sb[:].rearrange("p (e ht) -> p e ht", e=EPC), in_=b1[:])
        nc.gpsimd.dma_start(
            out=b2_sb[:].rearrange("p (e dt) -> p e dt", e=EPC), in_=b2[:])

        # All input transfers (x and weight chunks) form ONE stream in exact
        # consumption order, split across the sync HWDGE ring and the gpsimd
        # SWDGE queue by greedy byte-balancing weighted with each queue's
        # measured service rate (SWDGE drains ~1.3x faster when both are
        # saturated), so both rings finish each need-order window together.
        # The scalar ring carries only output stores: weight triggers there
        # would queue behind that expert's ACTs and lose prefetch lookahead.
        ring = [nc.sync, nc.gpsimd]
        g = [0]

        def stream_dma(out_ap, in_ap, nbytes):
            # First three transfers (x0 halves, W1e0 q0) ride sync: the
            # SWDGE queue's cold start would otherwise gate the first MMs.
            eng = nc.sync if g[0] < 3 else ring[g[0] % 2]
            g[0] += 1
            eng.dma_start(out=out_ap, in_=in_ap)

        HD = ND // 2

        def xload_half(e, h):
            # x in two dt-halves so the first L1 accumulation group can
            # start before the whole x tile has landed. x0 rides the sync
            # prefix; x1..x3 ride the scalar ring, which is otherwise idle
            # until the first stores (~21us) — this takes 1.6MB off the
            # weight streams and shrinks the early HBM deficit. bufs=EPC
            # keeps every x tile live so no x trigger ever waits on buffer
            # recycling (a waiting trigger would block the scalar FIFO
            # ahead of the ACTs it feeds -> deadlock).
            C = sizes[e]
            xt = xpool.tile([128, HD * S0], dt_a, tag=f"xt{h}", bufs=EPC)
            out_ap = xt[:, :HD * C].rearrange("p (dt t) -> p dt t", dt=HD)
            in_ap = (xT[h * HD * 128:(h + 1) * HD * 128, offs[e]:offs[e] + C]
                     .rearrange("(dt p) t -> p dt t", p=128))
            if e == 0:
                stream_dma(out_ap, in_ap, HD * C * 128 * 2)
            else:
                nc.scalar.dma_start(out=out_ap, in_=in_ap)
            return xt

        def wchunk(shape, tag, bufs, out_pat, in_ap):
            wt = wpool.tile(shape, dt_w, tag=tag, bufs=bufs)
            stream_dma(wt[:].rearrange(out_pat[0], **out_pat[1]), in_ap,
                       shape[0] * shape[1] * 2)
            return wt

        HND = ND // 2
        csz = NH // 4

        def w1chunk(e, q):
            # W1 quarters (fine grain so each L1 ht-group waits only for its
            # own 512KB slice; DMA-paced slips stay well under the HAM
            # re-throttle window)
            return wchunk(
                [128, csz * ND * 128], "w1q", 8,
                ("p (ht dt hi) -> p ht dt hi", dict(ht=csz, dt=ND)),
                w1[e, q * csz:(q + 1) * csz].rearrange("ht p dt hi -> p ht dt hi"),
            )

        xts = {}
        w1q, w2q = {}, {}
        # expert 0: interleave x halves with the first W1 quarters so the
        # first matmuls are gated on the minimum number of bytes.
        xts[0] = [None, None]
        xts[0][0] = xload_half(0, 0)
        w1q[0] = [w1chunk(0, 0)]
        xts[0][1] = xload_half(0, 1)
        w1q[0] += [w1chunk(0, q) for q in range(1, 4)]
        for e in range(EPC):
            if e > 0:
                w1q[e] = [w1chunk(e, q) for q in range(4)]
            w2q[e] = [wchunk(
                [128, HND * NH * 128], "w2h", 4,
                ("p (dt ht di) -> p dt ht di", dict(dt=HND, ht=NH)),
                w2[e, h * HND:(h + 1) * HND].rearrange("dt p ht di -> p dt ht di"),
            ) for h in range(2)]

        for e in range(EPC):
            C = sizes[e]
            off = offs[e]
            xt = xts[e]
            csz = NH // 4
            w1h = w1q[e]
            w2h = w2q[e]

            hts = []
            for ht in range(NH):
                wt = w1h[ht // csz]
                hoff = (ht % csz) * ND * 128
                ps = pp1.tile([128, S0], f32, tag="ps1")
                for dt_i in range(ND):
                    nc.tensor.matmul(
                        ps[:, :C],
                        wt[:, hoff + dt_i * 128: hoff + (dt_i + 1) * 128],
                        xt[dt_i // HD][:, (dt_i % HD) * C:(dt_i % HD + 1) * C],
                        start=(dt_i == 0),
                        stop=(dt_i == ND - 1),
                    )
                hsb = hpool.tile([128, S0], dt_a, tag="ht")
                nc.scalar.activation(
                    hsb[:, :C], ps[:, :C], gelu,
                    bias=b1_sb[:, e * NH + ht: e * NH + ht + 1],
                )
                hts.append(hsb)
            # Fetch the NEXT expert's x here, mid-stream on the scalar
            # FIFO (after this expert's ACTs): it transfers during this
            # expert's L2 window instead of stealing early HBM bandwidth
            # from the weight streams. bufs=EPC keeps the triggers
            # wait-free so they never block the FIFO ahead of later ACTs.
            if e + 1 < EPC:
                xts[e + 1] = [xload_half(e + 1, 0), xload_half(e + 1, 1)]
            ysb = ypool.tile([128, ND * S0], dt_y, tag="yt")
            for dt_i in range(ND):
                wt = w2h[dt_i // HND]
                doff = (dt_i % HND) * NH * 128
                ps2 = pp2.tile([128, S0], f32, tag="ps2")
                for ht in range(NH):
                    nc.tensor.matmul(
                        ps2[:, :C],
                        wt[:, doff + ht * 128: doff + (ht + 1) * 128],
                        hts[ht][:, :C],
                        start=(ht == 0),
                        stop=(ht == NH - 1),
                    )
                nc.vector.tensor_scalar_add(
                    ysb[:, dt_i * C:(dt_i + 1) * C], ps2[:, :C],
                    b2_sb[:, e * ND + dt_i: e * ND + dt_i + 1],
                )
                if dt_i % HND == HND - 1:
                    qi = dt_i // HND
                    r0, r1 = qi * HND * 128, (qi + 1) * HND * 128
                    nc.scalar.dma_start(
                        out=yT[r0:r1, off:off + C]
                        .rearrange("(dt p) t -> p dt t", p=128),
                        in_=ysb[:, qi * HND * C:(qi + 1) * HND * C]
                        .rearrange("p (dt t) -> p dt t", dt=HND),
                    )
    nc.compile()
    return nc


def _get_nc(sizes, dt_w, dt_a, dt_y):
    key = (sizes, dt_w, dt_a, dt_y)
    if key not in _NC_CACHE:
        _NC_CACHE[key] = _build_nc(sizes, dt_w, dt_a, dt_y)
    return _NC_CACHE[key]


def _np_dt(name):
    if name == "bfloat16":
        import ml_dtypes
        return np.dtype(ml_dtypes.bfloat16)
    return np.dtype(np.float32)


def _route(xf, Wg):
    """Replicates the reference gate exactly in f32 numpy."""
    logits = xf @ Wg                                     # [T, E]
    m = logits.max(-1, keepdims=True)
    ex = np.exp(logits - m)
    scores = ex / ex.sum(-1, keepdims=True)
    idx = np.argsort(-scores, axis=1, kind="stable")[:, :TOP_K]  # [T, k]
    tw = np.take_along_axis(scores, idx, 1)
    m2 = tw.max(-1, keepdims=True)
    e2 = np.exp(tw - m2)
    w = (e2 / e2.sum(-1, keepdims=True)).astype(np.float32)
    return idx.astype(np.int64), w


def kernel(x, Wg, W1, b1, W2, b2):
    global LAST_EXEC_TIME_NS
    from concourse import bass_utils

    dt_w, dt_a, dt_y = DT_W, DT_A, DT_Y
    orig_shape = x.shape
    x = np.asarray(x, dtype=np.float32)
    Wg = np.asarray(Wg, dtype=np.float32)
    W1 = np.asarray(W1, dtype=np.float32)
    b1 = np.asarray(b1, dtype=np.float32)
    W2 = np.asarray(W2, dtype=np.float32)
    b2 = np.asarray(b2, dtype=np.float32)
    xf = np.ascontiguousarray(x.reshape(-1, D))
    T = xf.shape[0]

    idx, w = _route(xf, Wg)

    # ---- dispatch: snake-balanced expert->core assignment, exact sizes
    counts = np.bincount(idx.reshape(-1), minlength=E)
    rank = np.argsort(-counts, kind="stable")             # expert ids, desc count
    core_of = np.zeros(E, np.int64)
    pos_of = np.zeros(E, np.int64)
    expert_at = np.zeros((N_CORES, EPC), np.int64)        # [core, pos] -> expert
    for k in range(EPC):
        for c in range(N_CORES):
            e = rank[k * N_CORES + (c if k % 2 == 0 else N_CORES - 1 - c)]
            core_of[e] = c
            pos_of[e] = k
            expert_at[c, k] = e
    sizes = tuple(
        int(-(-max(counts[expert_at[c, k]] for c in range(N_CORES)) // 8) * 8)
        for k in range(EPC)
    )
    offs = np.zeros(EPC, np.int64)
    for k in range(1, EPC):
        offs[k] = offs[k - 1] + sizes[k - 1]
    S = int(offs[-1] + sizes[-1])

    flat_e = idx.reshape(-1)                 # [k*T]
    flat_t = np.repeat(np.arange(T), TOP_K)
    order = np.argsort(flat_e, kind="stable")
    starts = np.zeros(E + 1, np.int64)
    starts[1:] = np.cumsum(counts)
    se = flat_e[order]
    pos = np.arange(TOP_K * T) - starts[se]  # slot within expert block
    core = core_of[se]
    col = offs[pos_of[se]] + pos             # column in that core's xT
    tok = flat_t[order]

    gidx = np.zeros((N_CORES, S), np.int64)
    for c in range(N_CORES):
        msel = core == c
        gidx[c, col[msel]] = tok[msel]

    np_w = _np_dt(dt_w)
    np_a = _np_dt(dt_a)
    xf_a = xf.astype(np_a, copy=False)
    # pre-tile weights: w1 -> [e, ht, p(d_in), dt, hi], w2 -> [e, dt, p(h_in), ht, di]
    W1t = np.ascontiguousarray(
        W1.reshape(E, ND, 128, NH, 128).transpose(0, 3, 2, 1, 4).astype(np_w, copy=False))
    W2t = np.ascontiguousarray(
        W2.reshape(E, NH, 128, ND, 128).transpose(0, 3, 2, 1, 4).astype(np_w, copy=False))
    # pre-transpose biases to [p, e, col_tile]
    b1t = np.ascontiguousarray(b1.reshape(E, NH, 128).transpose(2, 0, 1))
    b2t = np.ascontiguousarray(b2.reshape(E, ND, 128).transpose(2, 0, 1))

    in_maps = []
    for c in range(N_CORES):
        es = expert_at[c]
        in_maps.append({
            "xT": np.ascontiguousarray(xf_a[gidx[c]].T),
            "w1": np.ascontiguousarray(W1t[es]),
            "w2": np.ascontiguousarray(W2t[es]),
            "b1": np.ascontiguousarray(b1t[:, es]),
            "b2": np.ascontiguousarray(b2t[:, es]),
        })

    nc = _get_nc(sizes, dt_w, dt_a, dt_y)
    trace = os.environ.get("MOE_TRACE", "0") == "1"
    res = bass_utils.run_bass_kernel_spmd(
        nc, in_maps, core_ids=list(range(N_CORES)), trace=trace,
    )
    LAST_EXEC_TIME_NS = res.exec_time_ns

    # ---- combine: gather each (token, k) contribution, weight, and sum
    Ystack = np.stack(
        [res.results[c]["yT"].T.astype(np.float32) for c in range(N_CORES)])
    contrib = Ystack[core, col]              # [k*T, D] (sorted order)
    inv = np.empty_like(order)
    inv[order] = np.arange(TOP_K * T)
    contrib = contrib[inv].reshape(T, TOP_K, D)
    y = (contrib * w[:, :, None]).sum(1).astype(np.float32)
    return y.reshape(orig_shape)


# revision 41
# speedup vs baseline: 1.2013x; 1.1983x over previous
"""MoE MLP (E=32 experts, top-2, D=H=1024) on 8 Trainium2 NeuronCores.

Strategy (expert parallel, per sharding hint):
  * Host computes the (tiny) gate: softmax(x @ Wg), top-2, renormalized
    weights, and dispatches tokens per expert into per-expert token blocks,
    transposed to [D, tokens] (features on SBUF partitions, tokens on the
    matmul moving/free dimension). This is the sharding/all-to-all step.
  * Experts are assigned to cores in "snake" order of descending token
    count, so every core holds 4 experts whose block sizes match the
    per-position maximum; blocks are sized to the actual routed counts
    (rounded up to 8) instead of a uniform worst-case capacity.  SPMD
    requires one program for all cores, so position k on every core uses
    the same block size s_k = max over cores of that position's count.
  * Each core computes GELU(x W1 + b1) W2 + b2 for its experts' blocks.
  * Host combines with the top-2 gate weights (scatter-add).

Device kernel notes:
  * Weights are host-pre-tiled to [e, col_tile, partition, k_tile, 128] so
    each chunk streams in as one fully-contiguous DMA transfer.
  * All input transfers (x halves, W1 quarters, W2 halves) form one stream
    in exact consumption order, ping-ponged across the sync HWDGE ring and
    the gpsimd SWDGE queue with bounded (bufs) lookahead, so delivery
    tracks the need order at aggregate HBM rate (~345 GB/s/core, the
    binding constraint — this problem sits right at the compute/memory
    ridge). The scalar ring carries only output stores: weight triggers
    there would queue behind that expert's ACTs and lose all prefetch
    lookahead.
  * Output is written per half-expert in bf16, so the post-matmul tail
    (bias add + store) is short and overlaps the next tile's matmuls.
  * A chain of N=512 dummy matmuls at kernel start bridges the PE from
    t~7.5us until the first weight/x chunks land (~12.5us), keeping the
    HAM clock-gate warm through the handoff to real matmuls.
"""

import os
import sys
import numpy as np

for _p in ("/root/.axon_site/_ro/trn_rl_repo", "/opt/trn_rl_repo"):
    if _p not in sys.path and os.path.isdir(_p):
        sys.path.append(_p)

E, D, H = 32, 1024, 1024
TOP_K = 2
N_CORES = 8
EPC = E // N_CORES  # experts per core
ND = D // 128       # d 128-tiles
NH = H // 128       # h 128-tiles

# weight dtype, activation dtype (must both be 16-bit or both 32-bit)
DT_W = os.environ.get("MOE_DT_W", "bfloat16")
DT_A = os.environ.get("MOE_DT_A", "bfloat16")
DT_Y = os.environ.get("MOE_DT_Y", "bfloat16")
N_WARMUP_MM = int(os.environ.get("MOE_WARMUP", "16"))
WARMUP_N = int(os.environ.get("MOE_WARMUP_N", "512"))
WBUFS = int(os.environ.get("MOE_WBUFS", "6"))

LAST_EXEC_TIME_NS = None

_NC_CACHE = {}


def _build_nc(sizes, dt_w_name, dt_a_name, dt_y_name):
    import concourse.bass as bass  # noqa: F401
    import concourse.tile as tile
    from concourse import bacc, mybir
    from contextlib import ExitStack

    f32 = mybir.dt.float32
    dt_w = getattr(mybir.dt, dt_w_name)
    dt_a = getattr(mybir.dt, dt_a_name)
    dt_y = getattr(mybir.dt, dt_y_name)
    S = sum(sizes)

    nc = bacc.Bacc(
        "TRN2",
        target_bir_lowering=False,
        debug=False,
        enable_asserts=False,
        num_devices=N_CORES,
    )
    xT = nc.dram_tensor("xT", [D, S], dt_a, kind="ExternalInput").ap()
    # host-pre-tiled: w1[e, ht, p(=d_in), dt, hi], w2[e, dt, p(=h_in), ht, di]
    w1 = nc.dram_tensor("w1", [EPC, NH, 128, ND, 128], dt_w, kind="ExternalInput").ap()
    w2 = nc.dram_tensor("w2", [EPC, ND, 128, NH, 128], dt_w, kind="ExternalInput").ap()
    # host-pre-transposed biases: [p, e, col_tile]
    b1 = nc.dram_tensor("b1", [128, EPC, NH], f32, kind="ExternalInput").ap()
    b2 = nc.dram_tensor("b2", [128, EPC, ND], f32, kind="ExternalInput").ap()
    yT = nc.dram_tensor("yT", [D, S], dt_y, kind="ExternalOutput").ap()

    with tile.TileContext(nc) as tc, ExitStack() as ctx:
        wpool = ctx.enter_context(tc.tile_pool(name="w", bufs=3))
        xpool = ctx.enter_context(tc.tile_pool(name="x", bufs=EPC))
        hpool = ctx.enter_context(tc.tile_pool(name="h", bufs=NH))
        ypool = ctx.enter_context(tc.tile_pool(name="y", bufs=2))
        bpool = ctx.enter_context(tc.tile_pool(name="b", bufs=1))
        pp1 = ctx.enter_context(tc.tile_pool(name="ps1", bufs=4, space="PSUM"))
        pp2 = ctx.enter_context(tc.tile_pool(name="ps2", bufs=3, space="PSUM"))
        ppw = ctx.enter_context(tc.tile_pool(name="psw", bufs=1, space="PSUM"))

        # PE warm-up: dummy matmuls with no DMA dependency keep the PE
        # busy from t~0 so HAM un-throttles before the real matmuls.
        if N_WARMUP_MM:
            wu = bpool.tile([128, WARMUP_N], mybir.dt.bfloat16, tag="wu")
            nc.vector.memset(wu[:], 0.0)
            wups = ppw.tile([128, WARMUP_N], f32, tag="psw")
            for i in range(N_WARMUP_MM):
                nc.tensor.matmul(wups[:], wu[:, :128], wu[:],
                                 start=(i == 0), stop=(i == N_WARMUP_MM - 1))

        gelu = mybir.ActivationFunctionType.Gelu
        S0 = sizes[0]  # max block size; all tiles sized for it, sliced to C
        offs = [sum(sizes[:k]) for k in range(EPC)]

        # Biases first on gpsimd — tiny transfers that absorb the SWDGE
        # queue's ~2us descriptor-emission cold-start before its first
        # weight chunk (measured: moving them off gpsimd delays W1e0's
        # gpsimd-side quarters and stalls L1(e0) by ~3us).
        b1_sb = bpool.tile([128, EPC * NH], f32, tag="b1")
        b2_sb = bpool.tile([128, EPC * ND], f32, tag="b2")
        nc.gpsimd.dma_start(
            out=b1_sb[:].rearrange("p (e ht) -> p e ht", e=EPC), in_=b1[:])
        nc.gpsimd.dma_start(
            out=b2_sb[:].rearrange("p (e dt) -> p e dt", e=EPC), in_=b2[:])

        # All input transfers (x and weight chunks) form ONE stream in exact
        # consumption order, split across the sync HWDGE ring and the gpsimd
        # SWDGE queue by greedy byte-balancing weighted with each queue's
        # measured service rate (SWDGE drains ~1.3x faster when both are
        # saturated), so both rings finish each need-order window together.
        # The scalar ring carries only output stores: weight triggers there
        # would queue behind that expert's ACTs and lose prefetch lookahead.
        ring = [nc.sync, nc.gpsimd]
        g = [0]

        def stream_dma(out_ap, in_ap, nbytes):
            # First three transfers (x0 halves, W1e0 q0) ride sync: the
            # SWDGE queue's cold start would otherwise gate the first MMs.
            eng = nc.sync if g[0] < 3 else ring[g[0] % 2]
            g[0] += 1
            eng.dma_start(out=out_ap, in_=in_ap)

        HD = ND // 2

        def xload_half(e, h):
            # x in two dt-halves so the first L1 accumulation group can
            # start before the whole x tile has landed.
            C = sizes[e]
            xt = xpool.tile([128, HD * S0], dt_a, tag=f"xt{h}", bufs=2)
            stream_dma(
                xt[:, :HD * C].rearrange("p (dt t) -> p dt t", dt=HD),
                xT[h * HD * 128:(h + 1) * HD * 128, offs[e]:offs[e] + C]
                .rearrange("(dt p) t -> p dt t", p=128),
                HD * C * 128 * 2,
            )
            return xt

        def wchunk(shape, tag, bufs, out_pat, in_ap):
            wt = wpool.tile(shape, dt_w, tag=tag, bufs=bufs)
            stream_dma(wt[:].rearrange(out_pat[0], **out_pat[1]), in_ap,
                       shape[0] * shape[1] * 2)
            return wt

        HND = ND // 2
        csz = NH // 4

        def w1chunk(e, q):
            # W1 quarters (fine grain so each L1 ht-group waits only for its
            # own 512KB slice; DMA-paced slips stay well under the HAM
            # re-throttle window)
            return wchunk(
                [128, csz * ND * 128], "w1q", 8,
                ("p (ht dt hi) -> p ht dt hi", dict(ht=csz, dt=ND)),
                w1[e, q * csz:(q + 1) * csz].rearrange("ht p dt hi -> p ht dt hi"),
            )

        xts = {}
        w1q, w2q = {}, {}
        # expert 0: interleave x halves with the first W1 quarters so the
        # first matmuls are gated on the minimum number of bytes.
        xts[0] = [None, None]
        xts[0][0] = xload_half(0, 0)
        w1q[0] = [w1chunk(0, 0)]
        xts[0][1] = xload_half(0, 1)
        w1q[0] += [w1chunk(0, q) for q in range(1, 4)]
        for e in range(EPC):
            if e > 0:
                w1q[e] = [w1chunk(e, q) for q in range(4)]
            w2q[e] = [wchunk(
                [128, HND * NH * 128], "w2h", 4,
                ("p (dt ht di) -> p dt ht di", dict(dt=HND, ht=NH)),
                w2[e, h * HND:(h + 1) * HND].rearrange("dt p ht di -> p dt ht di"),
            ) for h in range(2)]
            if e + 1 < EPC:
                xts[e + 1] = [xload_half(e + 1, 0), xload_half(e + 1, 1)]

        for e in range(EPC):
            C = sizes[e]
            off = offs[e]
            xt = xts[e]
            csz = NH // 4
            w1h = w1q[e]
            w2h = w2q[e]

            hts = []
            for ht in range(NH):
                wt = w1h[ht // csz]
                hoff = (ht % csz) * ND * 128
                ps = pp1.tile([128, S0], f32, tag="ps1")
                for dt_i in range(ND):
                    nc.tensor.matmul(
                        ps[:, :C],
                        wt[:, hoff + dt_i * 128: hoff + (dt_i + 1) * 128],
                        xt[dt_i // HD][:, (dt_i % HD) * C:(dt_i % HD + 1) * C],
                        start=(dt_i == 0),
                        stop=(dt_i == ND - 1),
                    )
                hsb = hpool.tile([128, S0], dt_a, tag="ht")
                nc.scalar.activation(
                    hsb[:, :C], ps[:, :C], gelu,
                    bias=b1_sb[:, e * NH + ht: e * NH + ht + 1],
                )
                hts.append(hsb)
            ysb = ypool.tile([128, ND * S0], dt_y, tag="yt")
            for dt_i in range(ND):
                wt = w2h[dt_i // HND]
                doff = (dt_i % HND) * NH * 128
                ps2 = pp2.tile([128, S0], f32, tag="ps2")
                for ht in range(NH):
                    nc.tensor.matmul(
                        ps2[:, :C],
                        wt[:, doff + ht * 128: doff + (ht + 1) * 128],
                        hts[ht][:, :C],
                        start=(ht == 0),
                        stop=(ht == NH - 1),
                    )
                nc.vector.tensor_scalar_add(
                    ysb[:, dt_i * C:(dt_i + 1) * C], ps2[:, :C],
                    b2_sb[:, e * ND + dt_i: e * ND + dt_i + 1],
                )
                if dt_i % HND == HND - 1:
                    qi = dt_i // HND
                    r0, r1 = qi * HND * 128, (qi + 1) * HND * 128
                    nc.scalar.dma_start(
                        out=yT[r0:r1, off:off + C]
                        .rearrange("(dt p) t -> p dt t", p=128),
                        in_=ysb[:, qi * HND * C:(qi + 1) * HND * C]
                        .rearrange("p (dt t) -> p dt t", dt=HND),
                    )
    nc.compile()
    return nc


def _get_nc(sizes, dt_w, dt_a, dt_y):
    key = (sizes, dt_w, dt_a, dt_y)
    if key not in _NC_CACHE:
        _NC_CACHE[key] = _build_nc(sizes, dt_w, dt_a, dt_y)
    return _NC_CACHE[key]


def _np_dt(name):
    if name == "bfloat16":
        import ml_dtypes
        return np.dtype(ml_dtypes.bfloat16)
    return np.dtype(np.float32)


def _route(xf, Wg):
    """Replicates the reference gate exactly in f32 numpy."""
    logits = xf @ Wg                                     # [T, E]
    m = logits.max(-1, keepdims=True)
    ex = np.exp(logits - m)
    scores = ex / ex.sum(-1, keepdims=True)
    idx = np.argsort(-scores, axis=1, kind="stable")[:, :TOP_K]  # [T, k]
    tw = np.take_along_axis(scores, idx, 1)
    m2 = tw.max(-1, keepdims=True)
    e2 = np.exp(tw - m2)
    w = (e2 / e2.sum(-1, keepdims=True)).astype(np.float32)
    return idx.astype(np.int64), w


def kernel(x, Wg, W1, b1, W2, b2):
    global LAST_EXEC_TIME_NS
    from concourse import bass_utils

    dt_w, dt_a, dt_y = DT_W, DT_A, DT_Y
    orig_shape = x.shape
    x = np.asarray(x, dtype=np.float32)
    Wg = np.asarray(Wg, dtype=np.float32)
    W1 = np.asarray(W1, dtype=np.float32)
    b1 = np.asarray(b1, dtype=np.float32)
    W2 = np.asarray(W2, dtype=np.float32)
    b2 = np.asarray(b2, dtype=np.float32)
    xf = np.ascontiguousarray(x.reshape(-1, D))
    T = xf.shape[0]

    idx, w = _route(xf, Wg)

    # ---- dispatch: snake-balanced expert->core assignment, exact sizes
    counts = np.bincount(idx.reshape(-1), minlength=E)
    rank = np.argsort(-counts, kind="stable")             # expert ids, desc count
    core_of = np.zeros(E, np.int64)
    pos_of = np.zeros(E, np.int64)
    expert_at = np.zeros((N_CORES, EPC), np.int64)        # [core, pos] -> expert
    for k in range(EPC):
        for c in range(N_CORES):
            e = rank[k * N_CORES + (c if k % 2 == 0 else N_CORES - 1 - c)]
            core_of[e] = c
            pos_of[e] = k
            expert_at[c, k] = e
    sizes = tuple(
        int(-(-max(counts[expert_at[c, k]] for c in range(N_CORES)) // 8) * 8)
        for k in range(EPC)
    )
    offs = np.zeros(EPC, np.int64)
    for k in range(1, EPC):
        offs[k] = offs[k - 1] + sizes[k - 1]
    S = int(offs[-1] + sizes[-1])

    flat_e = idx.reshape(-1)                 # [k*T]
    flat_t = np.repeat(np.arange(T), TOP_K)
    order = np.argsort(flat_e, kind="stable")
    starts = np.zeros(E + 1, np.int64)
    starts[1:] = np.cumsum(counts)
    se = flat_e[order]
    pos = np.arange(TOP_K * T) - starts[se]  # slot within expert block
    core = core_of[se]
    col = offs[pos_of[se]] + pos             # column in that core's xT
    tok = flat_t[order]

    gidx = np.zeros((N_CORES, S), np.int64)
    for c in range(N_CORES):
        msel = core == c
        gidx[c, col[msel]] = tok[msel]

    np_w = _np_dt(dt_w)
    np_a = _np_dt(dt_a)
    xf_a = xf.astype(np_a, copy=False)
    # pre-tile weights: w1 -> [e, ht, p(d_in), dt, hi], w2 -> [e, dt, p(h_in), ht, di]
    W1t = np.ascontiguousarray(
        W1.reshape(E, ND, 128, NH, 128).transpose(0, 3, 2, 1, 4).astype(np_w, copy=False))
    W2t = np.ascontiguousarray(
        W2.reshape(E, NH, 128, ND, 128).transpose(0, 3, 2, 1, 4).astype(np_w, copy=False))
    # pre-transpose biases to [p, e, col_tile]
    b1t = np.ascontiguousarray(b1.reshape(E, NH, 128).transpose(2, 0, 1))
    b2t = np.ascontiguousarray(b2.reshape(E, ND, 128).transpose(2, 0, 1))

    in_maps = []
    for c in range(N_CORES):
        es = expert_at[c]
        in_maps.append({
            "xT": np.ascontiguousarray(xf_a[gidx[c]].T),
            "w1": np.ascontiguousarray(W1t[es]),
            "w2": np.ascontiguousarray(W2t[es]),
            "b1": np.ascontiguousarray(b1t[:, es]),
            "b2": np.ascontiguousarray(b2t[:, es]),
        })

    nc = _get_nc(sizes, dt_w, dt_a, dt_y)
    trace = os.environ.get("MOE_TRACE", "0") == "1"
    res = bass_utils.run_bass_kernel_spmd(
        nc, in_maps, core_ids=list(range(N_CORES)), trace=trace,
    )
    LAST_EXEC_TIME_NS = res.exec_time_ns

    # ---- combine: gather each (token, k) contribution, weight, and sum
    Ystack = np.stack(
        [res.results[c]["yT"].T.astype(np.float32) for c in range(N_CORES)])
    contrib = Ystack[core, col]              # [k*T, D] (sorted order)
    inv = np.empty_like(order)
    inv[order] = np.arange(TOP_K * T)
    contrib = contrib[inv].reshape(T, TOP_K, D)
    y = (contrib * w[:, :, None]).sum(1).astype(np.float32)
    return y.reshape(orig_shape)
